# revision 3
# baseline (speedup 1.0000x reference)
"""Channel-attention (CTrans2) Trainium2 kernel.

Math per batch d (D=8, C=512, HW=4096):
    q = x.reshape(C, HW)
    energy = q @ q.T                        # (C, C)
    en = rowmax(energy) - energy
    a1 = softmax(en, axis=-1)
    a2 = softmax(a1 + atten, axis=-1)
    out = a2.T @ q                          # (C, HW)
    result = gamma * out + x

Sharding: data-parallel over D, one batch per NeuronCore (8 cores).

Implementation notes:
  - Matmuls run in float32r (fp32 storage, ~11-bit-mantissa multiply, fp32
    accumulate) at full PE rate. q arrives via DMA from f32r-declared DRAM;
    qT is produced on-chip with PE transposes.
  - Softmax matches the reference's fp32 op order: en = (-energy) + rowmax,
    m = rowmax(en), exp(en - m) / sum.
"""

import numpy as np

import concourse.bacc as bacc
import concourse.tile as tile
import concourse.mybir as mybir
from concourse import masks
from concourse import bass_utils

D, C, HW = 8, 512, 4096
P = 128          # partitions
NB = HW // 512   # 8 column blocks of q
CB = C // P      # 4 channel blocks
KT = HW // P     # 32 transposed k-chunks

F32 = mybir.dt.float32
F32R = mybir.dt.float32r
AF = mybir.ActivationFunctionType
ALU = mybir.AluOpType
AX = mybir.AxisListType

_CACHE = {}


def _build():
    nc = bacc.Bacc("TRN2", target_bir_lowering=False, debug=False)

    x_d = nc.dram_tensor("x", [C, HW], F32R, kind="ExternalInput").ap()
    at_d = nc.dram_tensor("atten", [C, C], F32, kind="ExternalInput").ap()
    g_d = nc.dram_tensor("gamma_b", [P, 1], F32, kind="ExternalInput").ap()
    out_d = nc.dram_tensor("out", [C, HW], F32, kind="ExternalOutput").ap()

    with tile.TileContext(nc) as tc:
        with (
            tc.tile_pool(name="qp", bufs=1) as qp,
            tc.tile_pool(name="qtp", bufs=1) as qtp,
            tc.tile_pool(name="smax", bufs=2) as smax,
            tc.tile_pool(name="small", bufs=1) as small,
            tc.tile_pool(name="outp", bufs=3) as outp,
            tc.tile_pool(name="ps_tr", bufs=2, space="PSUM") as ps_tr,
            tc.tile_pool(name="ps_e", bufs=1, space="PSUM") as ps_e,
            tc.tile_pool(name="ps_o", bufs=2, space="PSUM") as ps_o,
        ):
            # ---- constants / small inputs
            gam = small.tile([P, 1], F32, tag="gam")
            nc.sync.dma_start(gam[:], g_d[:])
            ident = small.tile([P, P], F32, tag="ident")
            masks.make_identity(nc, ident[:])

            att_in = [small.tile([P, C], F32, tag=f"attin{ci}", name=f"attin{ci}")
                      for ci in range(CB)]
            for ci in range(CB):
                nc.sync.dma_start(att_in[ci][:], at_d[ci * P:(ci + 1) * P, :])

            # ---- load q tiles: q[ci][nb] is (128, 512) f32r
            q = [[qp.tile([P, 512], F32R, tag=f"q{ci}_{nb}", name=f"q{ci}_{nb}")
                  for nb in range(NB)] for ci in range(CB)]
            for nb in range(NB):
                for ci in range(CB):
                    nc.sync.dma_start(
                        q[ci][nb][:],
                        x_d[ci * P:(ci + 1) * P, nb * 512:(nb + 1) * 512])

            # ---- build qT: qt[k] is (128 n, 512 c) f32r, k in 0..31
            qt = [qtp.tile([P, C], F32R, tag=f"qt{k}", name=f"qt{k}") for k in range(KT)]
            for k in range(KT):
                ptr = ps_tr.tile([P, C], F32, tag="tr")
                for ci in range(CB):
                    src = q[ci][k // 4][:, (k % 4) * P:(k % 4 + 1) * P].bitcast(F32)
                    nc.tensor.transpose(ptr[:, ci * P:(ci + 1) * P], src, ident[:])
                nc.vector.tensor_copy(qt[k][:], ptr[:])

            # ---- mm1 + softmax chain per channel-block mi
            a2 = [smax.tile([P, C], F32R, tag=f"a2_{mi}", name=f"a2_{mi}") for mi in range(CB)]
            for mi in range(CB):
                pe = ps_e.tile([P, C], F32, tag=f"e{mi}")
                for k in range(KT):
                    nc.tensor.matmul(
                        pe[:], qt[k][:, mi * P:(mi + 1) * P], qt[k][:],
                        start=(k == 0), stop=(k == KT - 1))

                # softmax 1 (matches reference fp32 op order)
                rmax = smax.tile([P, 1], F32, tag="rmax")
                nc.vector.tensor_reduce(rmax[:], pe[:], axis=AX.X, op=ALU.max)
                en = smax.tile([P, C], F32, tag="en")
                nc.vector.tensor_scalar(
                    en[:], pe[:], -1.0, rmax[:], op0=ALU.mult, op1=ALU.add)
                nm = smax.tile([P, 1], F32, tag="nm")
                nc.vector.tensor_reduce(
                    nm[:], en[:], axis=AX.X, op=ALU.max, negate=True)
                e1 = smax.tile([P, C], F32, tag="e1")
                s1 = smax.tile([P, 1], F32, tag="s1")
                nc.scalar.activation(
                    e1[:], en[:], AF.Exp, bias=nm[:], scale=1.0, accum_out=s1[:])
                r1 = smax.tile([P, 1], F32, tag="r1")
                nc.vector.reciprocal(r1[:], s1[:])

                # z = a1 + atten ; softmax 2
                z = smax.tile([P, C], F32, tag="z")
                nc.vector.scalar_tensor_tensor(
                    z[:], e1[:], r1[:], att_in[mi][:], op0=ALU.mult, op1=ALU.add)
                nm2 = smax.tile([P, 1], F32, tag="nm2")
                nc.vector.tensor_reduce(
                    nm2[:], z[:], axis=AX.X, op=ALU.max, negate=True)
                e2 = smax.tile([P, C], F32, tag="e2")
                s2 = smax.tile([P, 1], F32, tag="s2")
                nc.scalar.activation(
                    e2[:], z[:], AF.Exp, bias=nm2[:], scale=1.0, accum_out=s2[:])
                r2 = smax.tile([P, 1], F32, tag="r2")
                nc.vector.reciprocal(r2[:], s2[:])
                nc.vector.tensor_scalar_mul(a2[mi][:], e2[:], r2[:])

            # ---- mm2: out[mj, nb] = sum_ki a2[ki][:, mj].T @ q[ki][nb]
            for mj in range(CB):
                for nb in range(NB):
                    po = ps_o.tile([P, 512], F32, tag="po")
                    for ki in range(CB):
                        nc.tensor.matmul(
                            po[:], a2[ki][:, mj * P:(mj + 1) * P], q[ki][nb][:],
                            start=(ki == 0), stop=(ki == CB - 1))
                    ot = outp.tile([P, 512], F32, tag="ot")
                    nc.vector.scalar_tensor_tensor(
                        ot[:], po[:], gam[:], q[mj][nb][:].bitcast(F32),
                        op0=ALU.mult, op1=ALU.add)
                    nc.sync.dma_start(
                        out_d[mj * P:(mj + 1) * P, nb * 512:(nb + 1) * 512], ot[:])

    nc.compile()
    return nc


def get_nc():
    if "nc" not in _CACHE:
        _CACHE["nc"] = _build()
    return _CACHE["nc"]


def make_in_maps(inputs):
    x, atten, gamma = inputs["x"], inputs["atten"], inputs["gamma"]
    gb = np.broadcast_to(np.asarray(gamma, np.float32).reshape(1, 1), (P, 1)).copy()
    return [
        {
            "x": np.ascontiguousarray(np.asarray(x[d], np.float32).reshape(C, HW)),
            "atten": np.ascontiguousarray(np.asarray(atten[d], np.float32)),
            "gamma_b": gb,
        }
        for d in range(D)
    ]


def kernel(x: np.ndarray, atten: np.ndarray, gamma: np.ndarray) -> np.ndarray:
    assert x.shape == (D, C, 64, 64) and atten.shape == (D, C, C)
    nc = get_nc()
    in_maps = make_in_maps({"x": x, "atten": atten, "gamma": gamma})
    res = bass_utils.run_bass_kernel_spmd(nc, in_maps, list(range(D)))
    out = np.stack([res.results[d]["out"] for d in range(D)])
    return out.reshape(D, C, 64, 64).astype(np.float32)
